# revision 20
# baseline (speedup 1.0000x reference)
"""Trainium2 Bass kernel for nn_DecomposedAttention (B=2,H=8,N=2048,D=64).

Algebra: the reference chain
    Qt  = Q^T
    QX  = Q @ Qt                      [N,N]
    KXT = (K @ Qt)^T = Q @ K^T        [N,N]
    VX  = V @ Qt / 64                 [N,N]
    out = QX @ (KXT @ VX)
collapses (every big factor is rank-D) to
    out = Q @ [ (Q^T Q) @ (K^T V) / 64 ] @ Q^T  =  Q @ M @ Q^T,   M: [64,64]
so per (b,h) the work is two 64x64 Gram matrices, a tiny GEMM, and one
[N,64] @ [64,N] outer-product GEMM streamed to HBM.  The kernel is purely
output-bandwidth bound (~16.8 MB fp32 per head).

Sharding: B*H = 16 head-pairs, 2 per core across 8 cores (pure data
parallelism, no communication).

Precision: the final big GEMM runs as bf16 hi/lo split (Q^T and W=M@Q^T are
each split into bf16 hi + bf16 lo, stacked along the 128-partition
contraction dim), so one pair of accumulating matmuls per output tile
computes (Qh+Ql)(Wh+Wl) exactly, ~1e-5 relative error at full bf16 PE
throughput.  All small GEMMs run in fp32.

Layout: inputs are DMA'd contiguously - partition p holds rows 16p..16p+15,
so slice [:, s, :] is rows {16p+s}.  Gram accumulation is row-order
agnostic; PE transposes of those slices scatter into Q^T with free-dim
stride 16, which engine copies handle at full rate.
"""

import numpy as np

import concourse.bass as bass
import concourse.mybir as mybir
from concourse import bacc, masks, tile
from concourse.bass_utils import run_bass_kernel_spmd

B, H, N, D = 2, 8, 2048, 64
SCALE = 64.0
N_CORES = 8
HPC = (B * H) // N_CORES  # heads per core = 2
NS = N // 128  # 16 slices / row-tiles per head

F32 = mybir.dt.float32
BF16 = mybir.dt.bfloat16

_CACHED = None


def _build_nc(repeat=1, internal_out=False):
    """repeat>1 wraps the whole computation in a hardware loop (used only for
    wall-clock benchmarking); internal_out redirects the big output to an
    Internal DRAM scratch so benchmark runs don't transfer 268 MB."""
    nc = bacc.Bacc("TRN2", target_bir_lowering=False, debug=False)

    q = nc.dram_tensor("q", [HPC, N, D], F32, kind="ExternalInput")
    k = nc.dram_tensor("k", [HPC, N, D], F32, kind="ExternalInput")
    v = nc.dram_tensor("v", [HPC, N, D], F32, kind="ExternalInput")
    if internal_out:
        o = nc.dram_tensor("oscratch", [HPC, N, N], F32, kind="Internal")
        nc.dram_tensor("bench_out", [1, 4], F32, kind="ExternalOutput")
    else:
        o = nc.dram_tensor("o", [HPC, N, N], F32, kind="ExternalOutput")

    with tile.TileContext(nc) as tc:
        with (
            tc.tile_pool(name="const", bufs=1) as constp,
            tc.tile_pool(name="qin", bufs=2) as qinp,
            tc.tile_pool(name="kvin", bufs=2) as kvinp,
            tc.tile_pool(name="small", bufs=2) as smallp,
            tc.tile_pool(name="qtp", bufs=2) as qtp,
            tc.tile_pool(name="stat", bufs=2) as statp,
            tc.tile_pool(name="stage", bufs=6) as stagep,
            tc.tile_pool(name="pss", bufs=3, space="PSUM") as pss,
            tc.tile_pool(name="wpool", bufs=2, space="PSUM") as wpool,
            tc.tile_pool(name="psb", bufs=3, space="PSUM") as psb,
        ):
            ident = constp.tile([128, 128], F32)
            masks.make_identity(nc, ident[:])

            # PE warm-up: keep TensorE busy through the HAM activity window
            # while the first input DMAs land, so real setup matmuls run at
            # 2.4 GHz instead of the cold 1.2 GHz.
            wps = psb.tile([128, 512], F32, tag="big")
            for i in range(7):
                nc.tensor.matmul(
                    wps[:, :128], ident[:], ident[:], start=True, stop=True
                )

            st = {}

            def loads(h):
                qc = qinp.tile([128, NS, D], F32, tag="qc")
                kc = kvinp.tile([128, NS, D], F32, tag="kc")
                vc = kvinp.tile([128, NS, D], F32, tag="vc")
                # partition p <- rows 16p..16p+15 (fully contiguous DMA)
                nc.sync.dma_start(qc[:], q[h].rearrange("(p s) d -> p s d", p=128))
                nc.sync.dma_start(kc[:], k[h].rearrange("(p s) d -> p s d", p=128))
                nc.sync.dma_start(vc[:], v[h].rearrange("(p s) d -> p s d", p=128))
                st[h] = dict(qc=qc, kc=kc, vc=vc)

            def setup(h):
                """Generator: per-head preprocessing, yields at cheap
                suspension points so head h+1's setup can interleave with
                head h's big loop."""
                d = st[h]
                qc, kc, vc = d["qc"], d["kc"], d["vc"]

                # Q^T fp32; qt3[dd, p, s] = Q[16p+s, dd], flat free idx = n
                qt3 = qtp.tile([64, 128, NS], F32, tag="qt")
                qtf = qt3.rearrange("d p s -> d (p s)")
                # stacked bf16 stationary: rows 0..63 = Qh^T, 64..127 = Ql^T
                qst = statp.tile([128, N], BF16, tag="qst")
                qlo = statp.tile([64, N], BF16, tag="qlo")

                # A = Q^T Q: only needs qc (earliest DMA arrival)
                a_ps = pss.tile([64, 512], F32, tag="pss")
                for s in range(NS):
                    nc.tensor.matmul(
                        a_ps[:, :64], qc[:, s, :], qc[:, s, :],
                        start=(s == 0), stop=(s == NS - 1),
                    )
                    if s % 8 == 7:
                        yield
                a_sb = smallp.tile([64, 64], F32, tag="a")
                nc.scalar.copy(a_sb[:], a_ps[:, :64])

                # transposes: 4 slices per PSUM bank, one batched evac each
                for g in range(NS // 4):
                    pt = pss.tile([64, 512], F32, tag="pss")
                    for j in range(4):
                        s = 4 * g + j
                        nc.tensor.transpose(
                            pt[:, 128 * j : 128 * (j + 1)], qc[:, s, :], ident[:]
                        )
                    # psum free layout is (s', p); target wants (p, s)
                    src = pt.rearrange("d (s p) -> d p s", s=4)
                    dst = qt3[:, :, 4 * g : 4 * (g + 1)]
                    if g % 2 == 0:
                        nc.scalar.copy(dst, src)
                    else:
                        nc.vector.tensor_copy(dst, src)
                    yield

                # bf16 hi/lo split of Q^T (chunked so the in-order DVE queue
                # never blocks a later evacuation for long); the
                # partition-crossing lo move rides the ACT HWDGE ring in two
                # halves so it overlaps the tail of the split
                for c in range(4):
                    sl = slice(512 * c, 512 * (c + 1))
                    nc.vector.tensor_copy(qst[0:64, sl], qtf[:, sl])
                    nc.vector.tensor_sub(qlo[:, sl], qtf[:, sl], qst[0:64, sl])
                    if c == 1:
                        nc.scalar.dma_start(qst[64:128, 0:1024], qlo[:, 0:1024])
                        yield
                nc.scalar.dma_start(qst[64:128, 1024:2048], qlo[:, 1024:2048])
                yield

                # C = K^T V (k/v DMAs have landed by now)
                c_ps = pss.tile([64, 512], F32, tag="pss")
                for s in range(NS):
                    nc.tensor.matmul(
                        c_ps[:, :64], kc[:, s, :], vc[:, s, :],
                        start=(s == 0), stop=(s == NS - 1),
                    )
                    if s % 8 == 7:
                        yield
                c_sb = smallp.tile([64, 64], F32, tag="c")
                nc.scalar.copy(c_sb[:], c_ps[:, :64])

                # Mt = C^T A / SCALE (= M^T, A symmetric); duplicate along
                # free dim so one matmul emits W on both partition halves
                mt_ps = pss.tile([64, 512], F32, tag="pss")
                nc.tensor.matmul(mt_ps[:, :64], c_sb[:], a_sb[:], start=True, stop=True)
                mt2 = smallp.tile([64, 128], F32, tag="mt2")
                nc.scalar.mul(mt2[:, 0:64], mt_ps[:, :64], 1.0 / SCALE)
                nc.scalar.mul(mt2[:, 64:128], mt_ps[:, :64], 1.0 / SCALE)
                yield

                # W = M @ Q^T on both partition halves; split hi/lo on evac.
                # 256-col chunks keep the MM -> ACT-cast -> DVE-sub pipeline
                # latency short (W readiness gates the whole big loop).
                whs = statp.tile([128, N], BF16, tag="whs")
                wls = statp.tile([128, N], BF16, tag="wls")
                for c in range(N // 512):
                    w_ps = wpool.tile([128, 512], F32, tag="w")
                    sl = slice(512 * c, 512 * (c + 1))
                    nc.tensor.matmul(
                        w_ps[:], mt2[:], qtf[:, sl], start=True, stop=True
                    )
                    nc.scalar.copy(whs[:, sl], w_ps[:])
                    nc.vector.tensor_sub(wls[:, sl], w_ps[:], whs[:, sl])
                    yield
                d["qst"], d["whs"], d["wls"] = qst, whs, wls

            def big_tile(h, t, split_dma=False):
                d = st[h]
                qst, whs, wls = d["qst"], d["whs"], d["wls"]
                stg = stagep.tile([128, N], F32, tag="stage")
                lhs = qst[:, 128 * t : 128 * (t + 1)]
                rows = slice(128 * t, 128 * (t + 1))
                for c in range(4):
                    pbig = psb.tile([128, 512], F32, tag="big")
                    col = slice(512 * c, 512 * (c + 1))
                    nc.tensor.matmul(pbig[:], lhs, whs[:, col], start=True, stop=False)
                    nc.tensor.matmul(pbig[:], lhs, wls[:, col], start=False, stop=True)
                    if c % 2 == 0:
                        nc.vector.tensor_copy(stg[:, col], pbig[:])
                    else:
                        nc.scalar.copy(stg[:, col], pbig[:])
                    if split_dma and c == 1:
                        nc.sync.dma_start(o[h, rows, 0:1024], stg[:, 0:1024])
                if split_dma:
                    nc.sync.dma_start(o[h, rows, 1024:2048], stg[:, 1024:2048])
                else:
                    nc.sync.dma_start(o[h, rows, :], stg[:])

            def drain(gen):
                if gen is not None:
                    for _ in gen:
                        pass

            def emit_all():
                loads(0)
                drain(setup(0))
                loads(1)
                nxt = setup(1)
                for t in range(NS):
                    big_tile(0, t)
                    for _ in range(3):
                        if nxt is not None and (
                            next(nxt, StopIteration) is StopIteration
                        ):
                            nxt = None
                drain(nxt)
                for t in range(NS):
                    big_tile(1, t)

            for _rep in range(repeat):
                emit_all()

    nc.compile()
    return nc


def _get_nc():
    global _CACHED
    if _CACHED is None:
        _CACHED = _build_nc()
    return _CACHED


def _run(Q, K, V, **spmd_kwargs):
    Q = np.ascontiguousarray(np.asarray(Q, dtype=np.float32).reshape(B * H, N, D))
    K = np.ascontiguousarray(np.asarray(K, dtype=np.float32).reshape(B * H, N, D))
    V = np.ascontiguousarray(np.asarray(V, dtype=np.float32).reshape(B * H, N, D))

    nc = _get_nc()
    in_maps = [
        {
            "q": Q[c * HPC : (c + 1) * HPC],
            "k": K[c * HPC : (c + 1) * HPC],
            "v": V[c * HPC : (c + 1) * HPC],
        }
        for c in range(N_CORES)
    ]
    res = run_bass_kernel_spmd(
        nc, in_maps, core_ids=list(range(N_CORES)), **spmd_kwargs
    )
    out = np.concatenate([res.results[c]["o"] for c in range(N_CORES)], axis=0)
    return out.reshape(B, H, N, N), res


def kernel(X=None, Q=None, K=None, V=None):
    out, _ = _run(Q, K, V)
    return out


# revision 22
# speedup vs baseline: 316.3454x; 316.3454x over previous
"""Trainium2 Bass kernel for nn_DecomposedAttention (B=2,H=8,N=2048,D=64).

Algebra: the reference chain
    Qt  = Q^T
    QX  = Q @ Qt                      [N,N]
    KXT = (K @ Qt)^T = Q @ K^T        [N,N]
    VX  = V @ Qt / 64                 [N,N]
    out = QX @ (KXT @ VX)
collapses (every big factor is rank-D) to
    out = Q @ [ (Q^T Q) @ (K^T V) / 64 ] @ Q^T  =  Q @ M @ Q^T,   M: [64,64]
so per (b,h) the work is two 64x64 Gram matrices, a tiny GEMM, and one
[N,64] @ [64,N] outer-product GEMM streamed to HBM.  The kernel is purely
output-bandwidth bound (~16.8 MB fp32 per head).

Sharding: B*H = 16 head-pairs, 2 per core across 8 cores (pure data
parallelism, no communication).

Precision: the final big GEMM runs as bf16 hi/lo split (Q^T and W=M@Q^T are
each split into bf16 hi + bf16 lo, stacked along the 128-partition
contraction dim), so one pair of accumulating matmuls per output tile
computes (Qh+Ql)(Wh+Wl) exactly, ~1e-5 relative error at full bf16 PE
throughput.  All small GEMMs run in fp32.

Layout: inputs are DMA'd contiguously - partition p holds rows 16p..16p+15,
so slice [:, s, :] is rows {16p+s}.  Gram accumulation is row-order
agnostic; PE transposes of those slices scatter into Q^T with free-dim
stride 16, which engine copies handle at full rate.
"""

import numpy as np

import concourse.bass as bass
import concourse.mybir as mybir
from concourse import bacc, masks, tile
from concourse.bass_utils import run_bass_kernel_spmd

B, H, N, D = 2, 8, 2048, 64
SCALE = 64.0
N_CORES = 8
HPC = (B * H) // N_CORES  # heads per core = 2
NS = N // 128  # 16 slices / row-tiles per head

F32 = mybir.dt.float32
BF16 = mybir.dt.bfloat16

_CACHED = None


def _build_nc(repeat=1, internal_out=False):
    """repeat>1 wraps the whole computation in a hardware loop (used only for
    wall-clock benchmarking); internal_out redirects the big output to an
    Internal DRAM scratch so benchmark runs don't transfer 268 MB."""
    nc = bacc.Bacc("TRN2", target_bir_lowering=False, debug=False)

    q = nc.dram_tensor("q", [HPC, N, D], F32, kind="ExternalInput")
    k = nc.dram_tensor("k", [HPC, N, D], F32, kind="ExternalInput")
    v = nc.dram_tensor("v", [HPC, N, D], F32, kind="ExternalInput")
    if internal_out:
        o = nc.dram_tensor("oscratch", [HPC, N, N], F32, kind="Internal")
        nc.dram_tensor("bench_out", [1, 4], F32, kind="ExternalOutput")
    else:
        o = nc.dram_tensor("o", [HPC, N, N], F32, kind="ExternalOutput")

    with tile.TileContext(nc) as tc:
        with (
            tc.tile_pool(name="const", bufs=1) as constp,
            tc.tile_pool(name="qin", bufs=2) as qinp,
            tc.tile_pool(name="kvin", bufs=2) as kvinp,
            tc.tile_pool(name="small", bufs=2) as smallp,
            tc.tile_pool(name="qtp", bufs=2) as qtp,
            tc.tile_pool(name="stat", bufs=2) as statp,
            tc.tile_pool(name="stage", bufs=8) as stagep,
            tc.tile_pool(name="pss", bufs=3, space="PSUM") as pss,
            tc.tile_pool(name="wpool", bufs=2, space="PSUM") as wpool,
            tc.tile_pool(name="psb", bufs=3, space="PSUM") as psb,
        ):
            ident = constp.tile([128, 128], F32)
            masks.make_identity(nc, ident[:])

            # PE warm-up: keep TensorE busy through the HAM activity window
            # while the first input DMAs land, so real setup matmuls run at
            # 2.4 GHz instead of the cold 1.2 GHz.
            wps = psb.tile([128, 512], F32, tag="big")
            for i in range(7):
                nc.tensor.matmul(
                    wps[:, :128], ident[:], ident[:], start=True, stop=True
                )

            st = {}

            def loads(h):
                qc = qinp.tile([128, NS, D], F32, tag="qc")
                kc = kvinp.tile([128, NS, D], F32, tag="kc")
                vc = kvinp.tile([128, NS, D], F32, tag="vc")
                # partition p <- rows 16p..16p+15 (fully contiguous DMA)
                nc.sync.dma_start(qc[:], q[h].rearrange("(p s) d -> p s d", p=128))
                nc.sync.dma_start(kc[:], k[h].rearrange("(p s) d -> p s d", p=128))
                nc.sync.dma_start(vc[:], v[h].rearrange("(p s) d -> p s d", p=128))
                st[h] = dict(qc=qc, kc=kc, vc=vc)

            def setup(h):
                """Generator: per-head preprocessing, yields at cheap
                suspension points so head h+1's setup can interleave with
                head h's big loop."""
                d = st[h]
                qc, kc, vc = d["qc"], d["kc"], d["vc"]

                # Q^T fp32; qt3[dd, p, s] = Q[16p+s, dd], flat free idx = n
                qt3 = qtp.tile([64, 128, NS], F32, tag="qt")
                qtf = qt3.rearrange("d p s -> d (p s)")
                # stacked bf16 stationary: rows 0..63 = Qh^T, 64..127 = Ql^T
                qst = statp.tile([128, N], BF16, tag="qst")
                qlo = statp.tile([64, N], BF16, tag="qlo")

                # A = Q^T Q: only needs qc (earliest DMA arrival)
                a_ps = pss.tile([64, 512], F32, tag="pss")
                for s in range(NS):
                    nc.tensor.matmul(
                        a_ps[:, :64], qc[:, s, :], qc[:, s, :],
                        start=(s == 0), stop=(s == NS - 1),
                    )
                    if s % 8 == 7:
                        yield
                a_sb = smallp.tile([64, 64], F32, tag="a")
                nc.scalar.copy(a_sb[:], a_ps[:, :64])

                # transposes: 4 slices per PSUM bank, one batched evac each
                for g in range(NS // 4):
                    pt = pss.tile([64, 512], F32, tag="pss")
                    for j in range(4):
                        s = 4 * g + j
                        nc.tensor.transpose(
                            pt[:, 128 * j : 128 * (j + 1)], qc[:, s, :], ident[:]
                        )
                    # psum free layout is (s', p); target wants (p, s)
                    src = pt.rearrange("d (s p) -> d p s", s=4)
                    dst = qt3[:, :, 4 * g : 4 * (g + 1)]
                    if g % 2 == 0:
                        nc.scalar.copy(dst, src)
                    else:
                        nc.vector.tensor_copy(dst, src)
                    yield

                # bf16 hi/lo split of Q^T (chunked so the in-order DVE queue
                # never blocks a later evacuation for long); the
                # partition-crossing lo move rides the ACT HWDGE ring in two
                # halves so it overlaps the tail of the split
                for c in range(4):
                    sl = slice(512 * c, 512 * (c + 1))
                    nc.vector.tensor_copy(qst[0:64, sl], qtf[:, sl])
                    nc.vector.tensor_sub(qlo[:, sl], qtf[:, sl], qst[0:64, sl])
                    if c == 1:
                        nc.scalar.dma_start(qst[64:128, 0:1024], qlo[:, 0:1024])
                        yield
                nc.scalar.dma_start(qst[64:128, 1024:2048], qlo[:, 1024:2048])
                yield

                # C = K^T V (k/v DMAs have landed by now)
                c_ps = pss.tile([64, 512], F32, tag="pss")
                for s in range(NS):
                    nc.tensor.matmul(
                        c_ps[:, :64], kc[:, s, :], vc[:, s, :],
                        start=(s == 0), stop=(s == NS - 1),
                    )
                    if s % 8 == 7:
                        yield
                c_sb = smallp.tile([64, 64], F32, tag="c")
                nc.scalar.copy(c_sb[:], c_ps[:, :64])

                # Mt = C^T A / SCALE (= M^T, A symmetric); duplicate along
                # free dim so one matmul emits W on both partition halves
                mt_ps = pss.tile([64, 512], F32, tag="pss")
                nc.tensor.matmul(mt_ps[:, :64], c_sb[:], a_sb[:], start=True, stop=True)
                mt2 = smallp.tile([64, 128], F32, tag="mt2")
                nc.scalar.mul(mt2[:, 0:64], mt_ps[:, :64], 1.0 / SCALE)
                nc.scalar.mul(mt2[:, 64:128], mt_ps[:, :64], 1.0 / SCALE)
                yield

                # W = M @ Q^T on both partition halves; split hi/lo straight
                # off PSUM (ACT casts the hi part, DVE subtracts for the lo
                # part) — W readiness gates the whole big loop.
                whs = statp.tile([128, N], BF16, tag="whs")
                wls = statp.tile([128, N], BF16, tag="wls")
                for c in range(N // 512):
                    w_ps = wpool.tile([128, 512], F32, tag="w")
                    sl = slice(512 * c, 512 * (c + 1))
                    nc.tensor.matmul(
                        w_ps[:], mt2[:], qtf[:, sl], start=True, stop=True
                    )
                    nc.scalar.copy(whs[:, sl], w_ps[:])
                    nc.vector.tensor_sub(wls[:, sl], w_ps[:], whs[:, sl])
                    yield
                d["qst"], d["whs"], d["wls"] = qst, whs, wls

            def big_tile(h, t, split_dma=False):
                d = st[h]
                qst, whs, wls = d["qst"], d["whs"], d["wls"]
                stg = stagep.tile([128, N], F32, tag="stage")
                lhs = qst[:, 128 * t : 128 * (t + 1)]
                rows = slice(128 * t, 128 * (t + 1))
                for c in range(4):
                    pbig = psb.tile([128, 512], F32, tag="big")
                    col = slice(512 * c, 512 * (c + 1))
                    nc.tensor.matmul(pbig[:], lhs, whs[:, col], start=True, stop=False)
                    nc.tensor.matmul(pbig[:], lhs, wls[:, col], start=False, stop=True)
                    if c % 2 == 0:
                        nc.vector.tensor_copy(stg[:, col], pbig[:])
                    else:
                        nc.scalar.copy(stg[:, col], pbig[:])
                    if split_dma and c == 1:
                        nc.sync.dma_start(o[h, rows, 0:1024], stg[:, 0:1024])
                if split_dma:
                    nc.sync.dma_start(o[h, rows, 1024:2048], stg[:, 1024:2048])
                else:
                    nc.sync.dma_start(o[h, rows, :], stg[:])

            def drain(gen):
                if gen is not None:
                    for _ in gen:
                        pass

            def emit_all():
                loads(0)
                drain(setup(0))
                loads(1)
                nxt = setup(1)
                for t in range(NS):
                    big_tile(0, t)
                    for _ in range(3):
                        if nxt is not None and (
                            next(nxt, StopIteration) is StopIteration
                        ):
                            nxt = None
                drain(nxt)
                for t in range(NS):
                    big_tile(1, t)

            for _rep in range(repeat):
                emit_all()

    nc.compile()
    return nc


def _get_nc():
    global _CACHED
    if _CACHED is None:
        _CACHED = _build_nc()
    return _CACHED


def _run(Q, K, V, **spmd_kwargs):
    Q = np.ascontiguousarray(np.asarray(Q, dtype=np.float32).reshape(B * H, N, D))
    K = np.ascontiguousarray(np.asarray(K, dtype=np.float32).reshape(B * H, N, D))
    V = np.ascontiguousarray(np.asarray(V, dtype=np.float32).reshape(B * H, N, D))

    nc = _get_nc()
    in_maps = [
        {
            "q": Q[c * HPC : (c + 1) * HPC],
            "k": K[c * HPC : (c + 1) * HPC],
            "v": V[c * HPC : (c + 1) * HPC],
        }
        for c in range(N_CORES)
    ]
    res = run_bass_kernel_spmd(
        nc, in_maps, core_ids=list(range(N_CORES)), **spmd_kwargs
    )
    out = np.concatenate([res.results[c]["o"] for c in range(N_CORES)], axis=0)
    return out.reshape(B, H, N, N), res


def kernel(X=None, Q=None, K=None, V=None):
    out, _ = _run(Q, K, V)
    return out


# revision 32
# speedup vs baseline: 321.7853x; 1.0172x over previous
"""Trainium2 Bass kernel for nn_DecomposedAttention (B=2,H=8,N=2048,D=64).

Algebra: the reference chain
    Qt  = Q^T
    QX  = Q @ Qt                      [N,N]
    KXT = (K @ Qt)^T = Q @ K^T        [N,N]
    VX  = V @ Qt / 64                 [N,N]
    out = QX @ (KXT @ VX)
collapses (every big factor is rank-D) to
    out = Q @ [ (Q^T Q) @ (K^T V) / 64 ] @ Q^T  =  Q @ M @ Q^T,   M: [64,64]
so per (b,h) the work is two 64x64 Gram matrices, a tiny GEMM, and one
[N,64] @ [64,N] outer-product GEMM streamed to HBM.  The kernel is purely
output-bandwidth bound (~16.8 MB fp32 per head).

Sharding: B*H = 16 head-pairs, 2 per core across 8 cores (pure data
parallelism, no communication).

Precision: the final big GEMM runs as bf16 hi/lo split (Q^T and W=M@Q^T are
each split into bf16 hi + bf16 lo, stacked along the 128-partition
contraction dim), so one pair of accumulating matmuls per output tile
computes (Qh+Ql)(Wh+Wl) exactly, ~1e-5 relative error at full bf16 PE
throughput.  All small GEMMs run in fp32.

Layout: inputs are DMA'd contiguously - partition p holds rows 16p..16p+15,
so slice [:, s, :] is rows {16p+s}.  Gram accumulation is row-order
agnostic; PE transposes of those slices scatter into Q^T with free-dim
stride 16, which engine copies handle at full rate.
"""

import numpy as np

import concourse.bass as bass
import concourse.mybir as mybir
from concourse import bacc, masks, tile
from concourse.bass_utils import run_bass_kernel_spmd

B, H, N, D = 2, 8, 2048, 64
SCALE = 64.0
N_CORES = 8
HPC = (B * H) // N_CORES  # heads per core = 2
NS = N // 128  # 16 slices / row-tiles per head

F32 = mybir.dt.float32
BF16 = mybir.dt.bfloat16

_CACHED = None


def _build_nc(repeat=1, internal_out=False):
    """repeat>1 wraps the whole computation in a hardware loop (used only for
    wall-clock benchmarking); internal_out redirects the big output to an
    Internal DRAM scratch so benchmark runs don't transfer 268 MB."""
    nc = bacc.Bacc("TRN2", target_bir_lowering=False, debug=False)

    q = nc.dram_tensor("q", [HPC, N, D], F32, kind="ExternalInput")
    k = nc.dram_tensor("k", [HPC, N, D], F32, kind="ExternalInput")
    v = nc.dram_tensor("v", [HPC, N, D], F32, kind="ExternalInput")
    if internal_out:
        o = nc.dram_tensor("oscratch", [HPC, N, N], F32, kind="Internal")
        nc.dram_tensor("bench_out", [1, 4], F32, kind="ExternalOutput")
    else:
        o = nc.dram_tensor("o", [HPC, N, N], F32, kind="ExternalOutput")

    with tile.TileContext(nc) as tc:
        with (
            tc.tile_pool(name="const", bufs=1) as constp,
            tc.tile_pool(name="qin", bufs=2) as qinp,
            tc.tile_pool(name="kvin", bufs=2) as kvinp,
            tc.tile_pool(name="small", bufs=2) as smallp,
            tc.tile_pool(name="qtp", bufs=2) as qtp,
            tc.tile_pool(name="stat", bufs=2) as statp,
            tc.tile_pool(name="stage", bufs=8) as stagep,
            tc.tile_pool(name="pss", bufs=3, space="PSUM") as pss,
            tc.tile_pool(name="wpool", bufs=2, space="PSUM") as wpool,
            tc.tile_pool(name="psb", bufs=3, space="PSUM") as psb,
        ):
            ident = constp.tile([128, 128], F32)
            masks.make_identity(nc, ident[:])

            # PE warm-up: keep TensorE busy through the HAM activity window
            # while the first input DMAs land, so real setup matmuls run at
            # 2.4 GHz instead of the cold 1.2 GHz.
            wps = psb.tile([128, 512], F32, tag="big")
            for i in range(7):
                nc.tensor.matmul(
                    wps[:, :128], ident[:], ident[:], start=True, stop=True
                )

            st = {}

            def loads(h):
                qc = qinp.tile([128, NS, D], F32, tag="qc")
                kc = kvinp.tile([128, NS, D], F32, tag="kc")
                vc = kvinp.tile([128, NS, D], F32, tag="vc")
                # partition p <- rows 16p..16p+15 (fully contiguous DMA)
                nc.sync.dma_start(qc[:], q[h].rearrange("(p s) d -> p s d", p=128))
                nc.sync.dma_start(kc[:], k[h].rearrange("(p s) d -> p s d", p=128))
                nc.sync.dma_start(vc[:], v[h].rearrange("(p s) d -> p s d", p=128))
                st[h] = dict(qc=qc, kc=kc, vc=vc)

            def setup(h):
                """Generator: per-head preprocessing, yields at cheap
                suspension points so head h+1's setup can interleave with
                head h's big loop."""
                d = st[h]
                qc, kc, vc = d["qc"], d["kc"], d["vc"]

                # Q^T fp32; qt3[dd, p, s] = Q[16p+s, dd], flat free idx = n
                qt3 = qtp.tile([64, 128, NS], F32, tag="qt")
                qtf = qt3.rearrange("d p s -> d (p s)")
                # stacked bf16 stationary: rows 0..63 = Qh^T, 64..127 = Ql^T
                qst = statp.tile([128, N], BF16, tag="qst")
                qlo = statp.tile([64, N], BF16, tag="qlo")

                # A = Q^T Q: only needs qc (earliest DMA arrival)
                a_ps = pss.tile([64, 512], F32, tag="pss")
                for s in range(NS):
                    nc.tensor.matmul(
                        a_ps[:, :64], qc[:, s, :], qc[:, s, :],
                        start=(s == 0), stop=(s == NS - 1),
                    )
                    if s % 8 == 7:
                        yield
                a_sb = smallp.tile([64, 64], F32, tag="a")
                nc.scalar.copy(a_sb[:], a_ps[:, :64])

                # transposes + C Grams interleaved on PE: both are ready just
                # after the k/v loads land, and Mt only waits on C's tail
                c_ps = pss.tile([64, 512], F32, tag="pss")
                for g in range(NS // 4):
                    pt = pss.tile([64, 512], F32, tag="pss")
                    for j in range(4):
                        s = 4 * g + j
                        nc.tensor.transpose(
                            pt[:, 128 * j : 128 * (j + 1)], qc[:, s, :], ident[:]
                        )
                    for j in range(4):
                        s = 4 * g + j
                        nc.tensor.matmul(
                            c_ps[:, :64], kc[:, s, :], vc[:, s, :],
                            start=(s == 0), stop=(s == NS - 1),
                        )
                    # psum free layout is (s', p); target wants (p, s)
                    src = pt.rearrange("d (s p) -> d p s", s=4)
                    dst = qt3[:, :, 4 * g : 4 * (g + 1)]
                    if g % 2 == 0:
                        nc.scalar.copy(dst, src)
                    else:
                        nc.vector.tensor_copy(dst, src)
                    yield
                c_sb = smallp.tile([64, 64], F32, tag="c")
                nc.scalar.copy(c_sb[:], c_ps[:, :64])

                # Mt = C^T A / SCALE (= M^T, A symmetric); duplicate along
                # free dim so one matmul emits W on both partition halves
                mt_ps = pss.tile([64, 512], F32, tag="pss")
                nc.tensor.matmul(mt_ps[:, :64], c_sb[:], a_sb[:], start=True, stop=True)
                mt2 = smallp.tile([64, 128], F32, tag="mt2")
                nc.scalar.mul(mt2[:, 0:64], mt_ps[:, :64], 1.0 / SCALE)
                nc.scalar.mul(mt2[:, 64:128], mt_ps[:, :64], 1.0 / SCALE)
                yield

                # W = M @ Q^T on both partition halves; split hi/lo straight
                # off PSUM (ACT casts the hi part, DVE subtracts for the lo
                # part) — W readiness gates the whole big loop.  The qst hi/lo
                # split rides along chunk-by-chunk so the in-order DVE queue
                # serves each wls as soon as its whs lands.
                whs = statp.tile([128, N], BF16, tag="whs")
                wls = statp.tile([128, N], BF16, tag="wls")
                for c in range(N // 512):
                    w_ps = wpool.tile([128, 512], F32, tag="w")
                    sl = slice(512 * c, 512 * (c + 1))
                    nc.tensor.matmul(
                        w_ps[:], mt2[:], qtf[:, sl], start=True, stop=True
                    )
                    nc.scalar.copy(whs[:, sl], w_ps[:])
                    nc.vector.tensor_sub(wls[:, sl], w_ps[:], whs[:, sl])
                    nc.vector.tensor_copy(qst[0:64, sl], qtf[:, sl])
                    nc.vector.tensor_sub(qlo[:, sl], qtf[:, sl], qst[0:64, sl])
                    if c == 1:
                        # partition-crossing lo move, first half (covers the
                        # stationary slices for row-tiles 0..7)
                        nc.scalar.dma_start(qst[64:128, 0:1024], qlo[:, 0:1024])
                    yield
                nc.scalar.dma_start(qst[64:128, 1024:2048], qlo[:, 1024:2048])
                d["qst"], d["whs"], d["wls"] = qst, whs, wls

            def big_tile(h, t, split_dma=False):
                d = st[h]
                qst, whs, wls = d["qst"], d["whs"], d["wls"]
                stg = stagep.tile([128, N], F32, tag="stage")
                lhs = qst[:, 128 * t : 128 * (t + 1)]
                rows = slice(128 * t, 128 * (t + 1))
                for c in range(4):
                    pbig = psb.tile([128, 512], F32, tag="big")
                    col = slice(512 * c, 512 * (c + 1))
                    nc.tensor.matmul(pbig[:], lhs, whs[:, col], start=True, stop=False)
                    nc.tensor.matmul(pbig[:], lhs, wls[:, col], start=False, stop=True)
                    if c % 2 == 0:
                        nc.vector.tensor_copy(stg[:, col], pbig[:])
                    else:
                        nc.scalar.copy(stg[:, col], pbig[:])
                    if split_dma and c == 1:
                        nc.sync.dma_start(o[h, rows, 0:1024], stg[:, 0:1024])
                if split_dma:
                    nc.sync.dma_start(o[h, rows, 1024:2048], stg[:, 1024:2048])
                else:
                    nc.sync.dma_start(o[h, rows, :], stg[:])

            def drain(gen):
                if gen is not None:
                    for _ in gen:
                        pass

            def emit_all():
                loads(0)
                drain(setup(0))
                loads(1)
                nxt = setup(1)
                for t in range(NS):
                    big_tile(0, t, split_dma=(t == 0))
                    if t < 4:
                        # keep head-1 setup out of the engine queues while
                        # head 0's first tiles are still latency-critical
                        continue
                    for _ in range(4):
                        if nxt is not None and (
                            next(nxt, StopIteration) is StopIteration
                        ):
                            nxt = None
                drain(nxt)
                for t in range(NS):
                    big_tile(1, t)

            for _rep in range(repeat):
                emit_all()

    nc.compile()
    return nc


def _get_nc():
    global _CACHED
    if _CACHED is None:
        _CACHED = _build_nc()
    return _CACHED


def _run(Q, K, V, **spmd_kwargs):
    Q = np.ascontiguousarray(np.asarray(Q, dtype=np.float32).reshape(B * H, N, D))
    K = np.ascontiguousarray(np.asarray(K, dtype=np.float32).reshape(B * H, N, D))
    V = np.ascontiguousarray(np.asarray(V, dtype=np.float32).reshape(B * H, N, D))

    nc = _get_nc()
    in_maps = [
        {
            "q": Q[c * HPC : (c + 1) * HPC],
            "k": K[c * HPC : (c + 1) * HPC],
            "v": V[c * HPC : (c + 1) * HPC],
        }
        for c in range(N_CORES)
    ]
    res = run_bass_kernel_spmd(
        nc, in_maps, core_ids=list(range(N_CORES)), **spmd_kwargs
    )
    out = np.concatenate([res.results[c]["o"] for c in range(N_CORES)], axis=0)
    return out.reshape(B, H, N, N), res


def kernel(X=None, Q=None, K=None, V=None):
    out, _ = _run(Q, K, V)
    return out
